# revision 1
# baseline (speedup 1.0000x reference)
"""ChiSquareLoss kernel for Trainium2 (8 NeuronCores, SPMD).

Problem (see reference): for each of B=16384 rows of a [B, 2048] f32 matrix,
build a 10-bin histogram between the row's min and max, then
chi2_row = sum_j (obs_j - e)^2 / (e + eps) with e = B/10, and return
mean(chi2_row).

Algorithm per row (each core handles B/8 = 2048 rows):
  searchsorted(boundaries, x, side='left') == #boundaries strictly below x,
  so with t = (x - mn) / (mx - mn) in [0,1] the bin index is
  #{k in 1..9 : t > k/10}; the histogram is recovered from cumulative counts
  c_k = #{x in row : t > k/10}:  obs_j = c_j - c_{j+1}, c_0 = 2048, c_10 = 0.

Engine split per [128, 2048] tile:
  DVE : row max / row min (tensor_scalar + max/min accumulator, 2x fp32),
        t = (x - mn) * (1/(mx-mn)) (tensor_scalar fused),
        counts k=1..5 (is_gt + add accumulator, 2x fp32)
  ACT : counts k=6..9 via Sign(t - k/10) + sum accumulator
        (sum sign = 2*c_k - 2048 up to measure-zero exact-hit elements)
Epilogue: convert ACT sums to counts, difference into obs, one ACT
Square(obs - e) pass with accumulator -> per-partition partial sums.
Host: total / (e + eps) / B.
"""

import numpy as np

_B_FULL = 16384
_D = 2048
_N_CORES = 8
_ROWS_PER_CORE = _B_FULL // _N_CORES  # 2048
_P = 128
_TILES = _ROWS_PER_CORE // _P  # 16
_BINS = 10
# reference: expected = f32(B/BINS); expected + 1e-8 rounds back to the same f32
_E_F32 = np.float32(_B_FULL / _BINS)  # 1638.4f

_CACHE = {}


def _build_program():
    import concourse.bacc as bacc
    import concourse.mybir as mybir
    import concourse.tile as tile

    f32 = mybir.dt.float32
    bf16 = mybir.dt.bfloat16
    Alu = mybir.AluOpType
    Act = mybir.ActivationFunctionType

    nc = bacc.Bacc(None, target_bir_lowering=False)
    x = nc.dram_tensor("x", [_ROWS_PER_CORE, _D], f32, kind="ExternalInput")
    out = nc.dram_tensor("partial", [_P, 1], f32, kind="ExternalOutput")

    T = _TILES
    with tile.TileContext(nc) as tc:
        with tc.tile_pool(name="singles", bufs=1) as singles, \
             tc.tile_pool(name="xp", bufs=3) as xpool, \
             tc.tile_pool(name="tp", bufs=3) as tpool, \
             tc.tile_pool(name="dscr", bufs=2) as dscr, \
             tc.tile_pool(name="ascr", bufs=2) as ascr, \
             tc.tile_pool(name="small", bufs=4) as small:

            # persistent accumulator arrays
            c_dve = singles.tile([_P, T * 6], f32)   # per tile: j=0 (=2048) and c_1..c_5
            c_act = singles.tile([_P, T * 4], f32)   # per tile: sign-sums for k=6..9
            consts = singles.tile([_P, 5], f32)      # ACT bias columns
            c_dve3 = c_dve[:].rearrange("p (t k) -> p t k", k=6)
            nc.vector.memset(c_dve3[:, :, 0:1], float(_D))  # c_0 = 2048
            for i, k in enumerate(range(6, 10)):
                nc.vector.memset(consts[:, i:i + 1], -k / 10.0)
            nc.vector.memset(consts[:, 4:5], -float(_E_F32))

            for t in range(T):
                xt = xpool.tile([_P, _D], f32, tag="xt")
                nc.sync.dma_start(out=xt[:], in_=x[t * _P:(t + 1) * _P, :])

                mx = small.tile([_P, 1], f32, tag="mx")
                mn = small.tile([_P, 1], f32, tag="mn")
                delta = small.tile([_P, 1], f32, tag="delta")
                inv = small.tile([_P, 1], f32, tag="inv")

                s_mm = dscr.tile([_P, _D], bf16, tag="dvescr")
                nc.vector.tensor_scalar(s_mm[:], xt[:], 1.0, None,
                                        Alu.mult, Alu.max, accum_out=mx[:])
                s_mm2 = dscr.tile([_P, _D], bf16, tag="dvescr")
                nc.vector.tensor_scalar(s_mm2[:], xt[:], 1.0, None,
                                        Alu.mult, Alu.min, accum_out=mn[:])
                nc.vector.tensor_tensor(out=delta[:], in0=mx[:], in1=mn[:],
                                        op=Alu.subtract)
                nc.vector.reciprocal(inv[:], delta[:])

                tt = tpool.tile([_P, _D], f32, tag="tt")
                nc.vector.tensor_scalar(tt[:], xt[:], mn[:], inv[:],
                                        Alu.subtract, Alu.mult)

                base_d = t * 6
                for k in range(1, 6):
                    s = dscr.tile([_P, _D], bf16, tag="dvescr")
                    nc.vector.tensor_scalar(
                        s[:], tt[:], k / 10.0, None, Alu.is_gt, Alu.add,
                        accum_out=c_dve[:, base_d + k:base_d + k + 1])
                base_a = t * 4
                for i, k in enumerate(range(6, 10)):
                    s = ascr.tile([_P, _D], bf16, tag="actscr")
                    nc.scalar.activation(
                        s[:], tt[:], Act.Sign, bias=consts[:, i:i + 1], scale=1.0,
                        accum_out=c_act[:, base_a + i:base_a + i + 1])

            # ---- epilogue ----
            conv = singles.tile([_P, T * 4], f32)    # ACT sums -> counts
            nc.vector.tensor_scalar(conv[:], c_act[:], 0.5, float(_D // 2),
                                    Alu.mult, Alu.add)
            conv3 = conv[:].rearrange("p (t k) -> p t k", k=4)
            obs = singles.tile([_P, T * 10], f32)
            obs3 = obs[:].rearrange("p (t j) -> p t j", j=10)
            # obs_j = c_j - c_{j+1}; c_10 = 0
            nc.vector.tensor_tensor(out=obs3[:, :, 0:5], in0=c_dve3[:, :, 0:5],
                                    in1=c_dve3[:, :, 1:6], op=Alu.subtract)
            nc.vector.tensor_tensor(out=obs3[:, :, 5:6], in0=c_dve3[:, :, 5:6],
                                    in1=conv3[:, :, 0:1], op=Alu.subtract)
            nc.vector.tensor_tensor(out=obs3[:, :, 6:9], in0=conv3[:, :, 0:3],
                                    in1=conv3[:, :, 1:4], op=Alu.subtract)
            nc.vector.tensor_copy(obs3[:, :, 9:10], conv3[:, :, 3:4])

            sq = singles.tile([_P, T * 10], f32)
            part = singles.tile([_P, 1], f32)
            nc.scalar.activation(sq[:], obs[:], Act.Square,
                                 bias=consts[:, 4:5], scale=1.0,
                                 accum_out=part[:])
            nc.sync.dma_start(out=out[:], in_=part[:])

    nc.compile()
    return nc


def _get_program():
    if "nc" not in _CACHE:
        _CACHE["nc"] = _build_program()
    return _CACHE["nc"]


def kernel(embeddings: np.ndarray) -> np.ndarray:
    from concourse.bass_utils import run_bass_kernel_spmd

    assert embeddings.shape == (_B_FULL, _D), embeddings.shape
    x = np.ascontiguousarray(embeddings, dtype=np.float32)
    nc = _get_program()
    in_maps = [
        {"x": x[c * _ROWS_PER_CORE:(c + 1) * _ROWS_PER_CORE]}
        for c in range(_N_CORES)
    ]
    res = run_bass_kernel_spmd(nc, in_maps, core_ids=list(range(_N_CORES)))
    total = np.float64(0.0)
    for r in res.results:
        total += r["partial"].astype(np.float64).sum()
    mean_chi2 = total / np.float64(_E_F32) / np.float64(_B_FULL)
    return np.float32(mean_chi2)


# revision 2
# speedup vs baseline: 1.3173x; 1.3173x over previous
"""ChiSquareLoss kernel for Trainium2 (8 NeuronCores, SPMD).

Problem (see reference): for each of B=16384 rows of a [B, 2048] f32 matrix,
build a 10-bin histogram between the row's min and max, then
chi2_row = sum_j (obs_j - e)^2 / (e + eps) with e = B/10, and return
mean(chi2_row).

Algorithm per row (each core handles B/8 = 2048 rows):
  searchsorted(boundaries, x, side='left') == #boundaries strictly below x,
  so the bin index is #{k in 1..9 : x > b_k}, b_k = mn + (mx-mn)*k/10.
  The histogram comes from cumulative counts c_k = #{x in row : x > b_k}:
  obs_j = c_j - c_{j+1}, c_0 = 2048, c_10 = 0.  This matches the reference
  bit-for-bit (b_k computed with the same fp32 operation order).

Engine split per [128, 2048] tile (accumulator ops run at 1x: ~2.2us each):
  DVE : row max, row min (tensor_scalar + max/min accumulator),
        boundary vectors (tiny), counts k=1..3 (is_gt + add accumulator)
  ACT : counts k=4..9 via Sign(x - b_k) + sum accumulator
Epilogue: convert ACT sign-sums to counts, difference into obs, one ACT
Square(obs - e) pass with accumulator -> per-partition partial sums.
Host: total / (e + eps) / B.
"""

import numpy as np

_B_FULL = 16384
_D = 2048
_N_CORES = 8
_ROWS_PER_CORE = _B_FULL // _N_CORES  # 2048
_P = 128
_TILES = _ROWS_PER_CORE // _P  # 16
_BINS = 10
# reference: expected = f32(B/BINS); expected + 1e-8 rounds back to the same f32
_E_F32 = np.float32(_B_FULL / _BINS)  # 1638.4f

_N_DVE = 3  # boundaries 1.._N_DVE counted on DVE, the rest on ACT

_CACHE = {}


def _build_program():
    import concourse.bacc as bacc
    import concourse.mybir as mybir
    import concourse.tile as tile

    f32 = mybir.dt.float32
    bf16 = mybir.dt.bfloat16
    Alu = mybir.AluOpType
    Act = mybir.ActivationFunctionType

    nc = bacc.Bacc(None, target_bir_lowering=False)
    x = nc.dram_tensor("x", [_ROWS_PER_CORE, _D], f32, kind="ExternalInput")
    out = nc.dram_tensor("partial", [_P, 1], f32, kind="ExternalOutput")

    T = _TILES
    nd = _N_DVE
    na = 9 - nd
    # fracs exactly as the reference: f32(k)/f32(10)
    fr = [float(np.float32(k) / np.float32(10.0)) for k in range(1, 10)]

    with tile.TileContext(nc) as tc:
        with tc.tile_pool(name="singles", bufs=1) as singles, \
             tc.tile_pool(name="xp", bufs=3) as xpool, \
             tc.tile_pool(name="dscr", bufs=2) as dscr, \
             tc.tile_pool(name="ascr", bufs=2) as ascr, \
             tc.tile_pool(name="small", bufs=4) as small:

            # persistent accumulators: c_dve: j=0 const + c_1..c_nd; c_act: k=nd+1..9
            c_dve = singles.tile([_P, T * (nd + 1)], f32)
            c_act = singles.tile([_P, T * na], f32)
            fracs = singles.tile([_P, 9], f32)       # k/10, k=1..9
            nfracs = singles.tile([_P, 9], f32)      # -k/10
            ebias = singles.tile([_P, 1], f32)       # -e for the Square pass
            c_dve3 = c_dve[:].rearrange("p (t k) -> p t k", k=nd + 1)
            nc.vector.memset(c_dve3[:, :, 0:1], float(_D))  # c_0 = 2048
            for i, f in enumerate(fr):
                nc.vector.memset(fracs[:, i:i + 1], f)
                nc.vector.memset(nfracs[:, i:i + 1], -f)
            nc.vector.memset(ebias[:], -float(_E_F32))

            for t in range(T):
                xt = xpool.tile([_P, _D], f32, tag="xt")
                nc.sync.dma_start(out=xt[:], in_=x[t * _P:(t + 1) * _P, :])

                mx = small.tile([_P, 1], f32, tag="mx")
                nm = small.tile([_P, 1], f32, tag="nm")      # -min
                delta = small.tile([_P, 1], f32, tag="delta")
                ndelta = small.tile([_P, 1], f32, tag="ndelta")
                bpos = small.tile([_P, 9], f32, tag="bpos")  # b_k
                bneg = small.tile([_P, 9], f32, tag="bneg")  # -b_k

                s_mm = dscr.tile([_P, _D], bf16, tag="dvescr")
                nc.vector.tensor_scalar(s_mm[:], xt[:], 1.0, None,
                                        Alu.mult, Alu.max, accum_out=mx[:])
                s_mm2 = dscr.tile([_P, _D], bf16, tag="dvescr")
                nc.vector.tensor_scalar(s_mm2[:], xt[:], -1.0, None,
                                        Alu.mult, Alu.max, accum_out=nm[:])
                # delta = mx - mn = mx + nm ; ndelta = -delta
                nc.vector.tensor_tensor(out=delta[:], in0=mx[:], in1=nm[:],
                                        op=Alu.add)
                nc.vector.tensor_scalar(ndelta[:], delta[:], -1.0, None, Alu.mult)
                # b_k = fl(fl(delta*frac_k) - nm) == mn + delta*frac_k (reference order)
                nc.vector.tensor_scalar(bpos[:], fracs[:], delta[:], nm[:],
                                        Alu.mult, Alu.subtract)
                # -b_k = fl(fl(-delta*frac_k) + nm)  (same rounding, negated)
                nc.vector.tensor_scalar(bneg[:], fracs[:], ndelta[:], nm[:],
                                        Alu.mult, Alu.add)

                base_d = t * (nd + 1)
                for k in range(1, nd + 1):
                    s = dscr.tile([_P, _D], bf16, tag="dvescr")
                    nc.vector.tensor_scalar(
                        s[:], xt[:], bpos[:, k - 1:k], None, Alu.is_gt, Alu.add,
                        accum_out=c_dve[:, base_d + k:base_d + k + 1])
                base_a = t * na
                for i, k in enumerate(range(nd + 1, 10)):
                    s = ascr.tile([_P, _D], bf16, tag="actscr")
                    nc.scalar.activation(
                        s[:], xt[:], Act.Sign, bias=bneg[:, k - 1:k], scale=1.0,
                        accum_out=c_act[:, base_a + i:base_a + i + 1])

            # ---- epilogue ----
            conv = singles.tile([_P, T * na], f32)    # ACT sign-sums -> counts
            nc.vector.tensor_scalar(conv[:], c_act[:], 0.5, float(_D // 2),
                                    Alu.mult, Alu.add)
            conv3 = conv[:].rearrange("p (t k) -> p t k", k=na)
            obs = singles.tile([_P, T * 10], f32)
            obs3 = obs[:].rearrange("p (t j) -> p t j", j=10)
            # obs_j = c_j - c_{j+1}; c_10 = 0
            nc.vector.tensor_tensor(out=obs3[:, :, 0:nd], in0=c_dve3[:, :, 0:nd],
                                    in1=c_dve3[:, :, 1:nd + 1], op=Alu.subtract)
            nc.vector.tensor_tensor(out=obs3[:, :, nd:nd + 1],
                                    in0=c_dve3[:, :, nd:nd + 1],
                                    in1=conv3[:, :, 0:1], op=Alu.subtract)
            nc.vector.tensor_tensor(out=obs3[:, :, nd + 1:9],
                                    in0=conv3[:, :, 0:na - 1],
                                    in1=conv3[:, :, 1:na], op=Alu.subtract)
            nc.vector.tensor_copy(obs3[:, :, 9:10], conv3[:, :, na - 1:na])

            sq = singles.tile([_P, T * 10], f32)
            part = singles.tile([_P, 1], f32)
            nc.scalar.activation(sq[:], obs[:], Act.Square,
                                 bias=ebias[:], scale=1.0,
                                 accum_out=part[:])
            nc.sync.dma_start(out=out[:], in_=part[:])

    nc.compile()
    return nc


def _get_program():
    if "nc" not in _CACHE:
        _CACHE["nc"] = _build_program()
    return _CACHE["nc"]


def kernel(embeddings: np.ndarray) -> np.ndarray:
    from concourse.bass_utils import run_bass_kernel_spmd

    assert embeddings.shape == (_B_FULL, _D), embeddings.shape
    x = np.ascontiguousarray(embeddings, dtype=np.float32)
    nc = _get_program()
    in_maps = [
        {"x": x[c * _ROWS_PER_CORE:(c + 1) * _ROWS_PER_CORE]}
        for c in range(_N_CORES)
    ]
    res = run_bass_kernel_spmd(nc, in_maps, core_ids=list(range(_N_CORES)))
    total = np.float64(0.0)
    for r in res.results:
        total += r["partial"].astype(np.float64).sum()
    mean_chi2 = total / np.float64(_E_F32) / np.float64(_B_FULL)
    return np.float32(mean_chi2)


# revision 3
# speedup vs baseline: 1.3783x; 1.0463x over previous
"""ChiSquareLoss kernel for Trainium2 (8 NeuronCores, SPMD).

Problem (see reference): for each of B=16384 rows of a [B, 2048] f32 matrix,
build a 10-bin histogram between the row's min and max, then
chi2_row = sum_j (obs_j - e)^2 / (e + eps) with e = B/10, and return
mean(chi2_row).

Algorithm per row (each core handles B/8 = 2048 rows):
  searchsorted(boundaries, x, side='left') == #boundaries strictly below x,
  so the bin index is #{k in 1..9 : x > b_k}, b_k = mn + (mx-mn)*k/10.
  The histogram comes from cumulative counts c_k = #{x in row : x > b_k}:
  obs_j = c_j - c_{j+1}, c_0 = 2048, c_10 = 0.  b_k is computed with the
  reference's fp32 operation order, so counts match bit-for-bit.

Accumulated (reduction) ops run at 1x on DVE/ACT (~2.2us / [128,2048] tile),
so the kernel packs TWO counts per accumulated pass where it can:
  pair(lo,hi):  mask = (x > b_hi)*4096   (plain tensor_scalar, 2x mode)
                acc  = sum((x > b_lo) + mask)  (scalar_tensor_tensor, 1x)
  -> acc = c_lo + 4096*c_hi, exact in fp32 (c <= 2048, sum < 2^24).
Engine split per [128, 2048] tile:
  DVE : row max, row min (tensor_scalar + max/min accumulator),
        boundary vectors (tiny), pair-counts for k in {1,2} and {3,4}
  ACT : counts k=5..9 via Sign(x - b_k) + sum accumulator
Epilogue: unpack pairs (floor via the 2^23 magic constant), convert ACT
sign-sums to counts, difference into obs, one ACT Square(obs - e) pass with
accumulator -> per-partition partial sums.  Host: total / (e + eps) / B.
"""

import numpy as np

_B_FULL = 16384
_D = 2048
_N_CORES = 8
_ROWS_PER_CORE = _B_FULL // _N_CORES  # 2048
_P = 128
_TILES = _ROWS_PER_CORE // _P  # 16
_BINS = 10
# reference: expected = f32(B/BINS); expected + 1e-8 rounds back to the same f32
_E_F32 = np.float32(_B_FULL / _BINS)  # 1638.4f

_PAIRS = [(1, 2), (3, 4)]   # DVE pair-counted boundaries
_ACT_KS = [5, 6, 7, 8, 9]   # ACT sign-counted boundaries
_MAGIC = float(np.float32(2 ** 23 + 2 ** 22))  # round-to-int magic for fp32

_CACHE = {}


def _build_program():
    import concourse.bacc as bacc
    import concourse.mybir as mybir
    import concourse.tile as tile

    f32 = mybir.dt.float32
    bf16 = mybir.dt.bfloat16
    Alu = mybir.AluOpType
    Act = mybir.ActivationFunctionType

    nc = bacc.Bacc(None, target_bir_lowering=False)
    x = nc.dram_tensor("x", [_ROWS_PER_CORE, _D], f32, kind="ExternalInput")
    out = nc.dram_tensor("partial", [_P, 1], f32, kind="ExternalOutput")

    T = _TILES
    npair = len(_PAIRS)
    na = len(_ACT_KS)
    # fracs exactly as the reference: f32(k)/f32(10)
    fr = [float(np.float32(k) / np.float32(10.0)) for k in range(1, 10)]

    with tile.TileContext(nc) as tc:
        with tc.tile_pool(name="singles", bufs=1) as singles, \
             tc.tile_pool(name="xp", bufs=3) as xpool, \
             tc.tile_pool(name="dscr", bufs=2) as dscr, \
             tc.tile_pool(name="mscr", bufs=2) as mscr, \
             tc.tile_pool(name="pscr", bufs=2) as pscr, \
             tc.tile_pool(name="ascr", bufs=2) as ascr, \
             tc.tile_pool(name="small", bufs=4) as small:

            # persistent accumulators
            pairacc = singles.tile([_P, T * npair], f32)  # c_lo + 4096*c_hi
            sgnacc = singles.tile([_P, T * na], f32)      # ACT sign sums
            c_all = singles.tile([_P, T * 11], f32)       # c_0..c_10 per tile
            fracs = singles.tile([_P, 9], f32)            # k/10
            nfracs = singles.tile([_P, 9], f32)           # -k/10
            ebias = singles.tile([_P, 1], f32)            # -e
            c3 = c_all[:].rearrange("p (t k) -> p t k", k=11)
            nc.vector.memset(c3[:, :, 0:1], float(_D))    # c_0 = 2048
            nc.vector.memset(c3[:, :, 10:11], 0.0)        # c_10 = 0
            for i, f in enumerate(fr):
                nc.vector.memset(fracs[:, i:i + 1], f)
                nc.vector.memset(nfracs[:, i:i + 1], -f)
            nc.vector.memset(ebias[:], -float(_E_F32))

            for t in range(T):
                xt = xpool.tile([_P, _D], f32, tag="xt")
                nc.sync.dma_start(out=xt[:], in_=x[t * _P:(t + 1) * _P, :])

                mx = small.tile([_P, 1], f32, tag="mx")
                nm = small.tile([_P, 1], f32, tag="nm")      # -min
                delta = small.tile([_P, 1], f32, tag="delta")
                bpos = small.tile([_P, 9], f32, tag="bpos")  # b_k
                bneg = small.tile([_P, 9], f32, tag="bneg")  # -b_k

                s_mm = dscr.tile([_P, _D], bf16, tag="dvescr")
                nc.vector.tensor_scalar(s_mm[:], xt[:], 1.0, None,
                                        Alu.mult, Alu.max, accum_out=mx[:])
                s_mm2 = dscr.tile([_P, _D], bf16, tag="dvescr")
                nc.vector.tensor_scalar(s_mm2[:], xt[:], -1.0, None,
                                        Alu.mult, Alu.max, accum_out=nm[:])
                # delta = mx + nm = mx - mn
                nc.vector.tensor_tensor(out=delta[:], in0=mx[:], in1=nm[:],
                                        op=Alu.add)
                # b_k = fl(fl(delta*frac_k) - nm)  (reference rounding order)
                nc.vector.tensor_scalar(bpos[:], fracs[:], delta[:], nm[:],
                                        Alu.mult, Alu.subtract)
                # -b_k = fl(fl(-frac_k*delta) + nm)
                nc.vector.tensor_scalar(bneg[:], nfracs[:], delta[:], nm[:],
                                        Alu.mult, Alu.add)

                for pi, (lo, hi) in enumerate(_PAIRS):
                    mhi = mscr.tile([_P, _D], f32, tag="mask")
                    nc.vector.tensor_scalar(mhi[:], xt[:], bpos[:, hi - 1:hi],
                                            4096.0, Alu.is_gt, Alu.mult)
                    sp = pscr.tile([_P, _D], f32, tag="pair")
                    col = t * npair + pi
                    nc.vector.scalar_tensor_tensor(
                        out=sp[:], in0=xt[:], scalar=bpos[:, lo - 1:lo],
                        in1=mhi[:], op0=Alu.is_gt, op1=Alu.add,
                        accum_out=pairacc[:, col:col + 1])
                for i, k in enumerate(_ACT_KS):
                    s = ascr.tile([_P, _D], bf16, tag="actscr")
                    nc.scalar.activation(
                        s[:], xt[:], Act.Sign, bias=bneg[:, k - 1:k], scale=1.0,
                        accum_out=sgnacc[:, t * na + i:t * na + i + 1])

            # ---- epilogue ----
            # ACT sign-sums -> counts, scattered into c_all cols 5..9
            conv = singles.tile([_P, T * na], f32)
            nc.vector.tensor_scalar(conv[:], sgnacc[:], 0.5, float(_D // 2),
                                    Alu.mult, Alu.add)
            conv3 = conv[:].rearrange("p (t k) -> p t k", k=na)
            nc.vector.tensor_copy(c3[:, :, 5:10], conv3[:, :, 0:na])
            # unpack pairs: c_hi = round(pairacc/4096 - ~0.25) via magic, exact
            chi = singles.tile([_P, T * npair], f32)
            clo = singles.tile([_P, T * npair], f32)
            nc.vector.tensor_scalar(chi[:], pairacc[:], float(2.0 ** -12),
                                    _MAGIC, Alu.mult, Alu.add)
            nc.vector.tensor_scalar(chi[:], chi[:], -_MAGIC, None, Alu.add)
            nc.vector.scalar_tensor_tensor(
                out=clo[:], in0=chi[:], scalar=-4096.0, in1=pairacc[:],
                op0=Alu.mult, op1=Alu.add)
            chi3 = chi[:].rearrange("p (t k) -> p t k", k=npair)
            clo3 = clo[:].rearrange("p (t k) -> p t k", k=npair)
            for pi, (lo, hi) in enumerate(_PAIRS):
                nc.vector.tensor_copy(c3[:, :, lo:lo + 1], clo3[:, :, pi:pi + 1])
                nc.vector.tensor_copy(c3[:, :, hi:hi + 1], chi3[:, :, pi:pi + 1])
            # obs_j = c_j - c_{j+1}
            obs = singles.tile([_P, T * 10], f32)
            obs3 = obs[:].rearrange("p (t j) -> p t j", j=10)
            nc.vector.tensor_tensor(out=obs3[:, :, 0:10], in0=c3[:, :, 0:10],
                                    in1=c3[:, :, 1:11], op=Alu.subtract)

            sq = singles.tile([_P, T * 10], f32)
            part = singles.tile([_P, 1], f32)
            nc.scalar.activation(sq[:], obs[:], Act.Square,
                                 bias=ebias[:], scale=1.0,
                                 accum_out=part[:])
            nc.sync.dma_start(out=out[:], in_=part[:])

    nc.compile()
    return nc


def _get_program():
    if "nc" not in _CACHE:
        _CACHE["nc"] = _build_program()
    return _CACHE["nc"]


def kernel(embeddings: np.ndarray) -> np.ndarray:
    from concourse.bass_utils import run_bass_kernel_spmd

    assert embeddings.shape == (_B_FULL, _D), embeddings.shape
    x = np.ascontiguousarray(embeddings, dtype=np.float32)
    nc = _get_program()
    in_maps = [
        {"x": x[c * _ROWS_PER_CORE:(c + 1) * _ROWS_PER_CORE]}
        for c in range(_N_CORES)
    ]
    res = run_bass_kernel_spmd(nc, in_maps, core_ids=list(range(_N_CORES)))
    total = np.float64(0.0)
    for r in res.results:
        total += r["partial"].astype(np.float64).sum()
    mean_chi2 = total / np.float64(_E_F32) / np.float64(_B_FULL)
    return np.float32(mean_chi2)


# revision 4
# speedup vs baseline: 1.3925x; 1.0103x over previous
"""ChiSquareLoss kernel for Trainium2 (8 NeuronCores, SPMD).

Problem (see reference): for each of B=16384 rows of a [B, 2048] f32 matrix,
build a 10-bin histogram between the row's min and max, then
chi2_row = sum_j (obs_j - e)^2 / (e + eps) with e = B/10, and return
mean(chi2_row).

Algorithm per row (each core handles B/8 = 2048 rows):
  searchsorted(boundaries, x, side='left') == #boundaries strictly below x,
  so the bin index is #{k in 1..9 : x > b_k}, b_k = mn + (mx-mn)*k/10.
  The histogram comes from cumulative counts c_k = #{x in row : x > b_k}:
  obs_j = c_j - c_{j+1}, c_0 = 2048, c_10 = 0.  b_k is computed with the
  reference's fp32 operation order, so counts match bit-for-bit.

Accumulated (reduction) ops run at 1x on DVE/ACT (~2.2us / [128,2048] tile),
so the kernel packs TWO counts per accumulated pass where it can:
  pair(lo,hi):  mask = (x > b_hi)*4096   (plain tensor_scalar, 2x mode)
                acc  = sum((x > b_lo) + mask)  (scalar_tensor_tensor, 1x)
  -> acc = c_lo + 4096*c_hi, exact in fp32 (c <= 2048, sum < 2^24).
Engine split per [128, 2048] tile:
  DVE : row max, row min (tensor_scalar + max/min accumulator),
        boundary vectors (tiny), pair-counts for k in {1,2} and {3,4}
  ACT : counts k=5..9 via Sign(x - b_k) + sum accumulator
Epilogue: unpack pairs (floor via the 2^23 magic constant), convert ACT
sign-sums to counts, difference into obs, one ACT Square(obs - e) pass with
accumulator -> per-partition partial sums.  Host: total / (e + eps) / B.
"""

import numpy as np

_B_FULL = 16384
_D = 2048
_N_CORES = 8
_ROWS_PER_CORE = _B_FULL // _N_CORES  # 2048
_P = 128
_TILES = _ROWS_PER_CORE // _P  # 16
_BINS = 10
# reference: expected = f32(B/BINS); expected + 1e-8 rounds back to the same f32
_E_F32 = np.float32(_B_FULL / _BINS)  # 1638.4f

_PAIRS = [(1, 2), (3, 4)]   # DVE pair-counted boundaries
_ACT_KS = [5, 6, 7, 8, 9]   # ACT sign-counted boundaries
_MAGIC = float(np.float32(2 ** 23 + 2 ** 22))  # round-to-int magic for fp32

_CACHE = {}


def _build_program():
    import concourse.bacc as bacc
    import concourse.mybir as mybir
    import concourse.tile as tile

    f32 = mybir.dt.float32
    bf16 = mybir.dt.bfloat16
    Alu = mybir.AluOpType
    Act = mybir.ActivationFunctionType

    nc = bacc.Bacc(None, target_bir_lowering=False)
    x = nc.dram_tensor("x", [_ROWS_PER_CORE, _D], f32, kind="ExternalInput")
    out = nc.dram_tensor("partial", [_P, 1], f32, kind="ExternalOutput")

    T = _TILES
    npair = len(_PAIRS)
    na = len(_ACT_KS)
    # fracs exactly as the reference: f32(k)/f32(10)
    fr = [float(np.float32(k) / np.float32(10.0)) for k in range(1, 10)]

    with tile.TileContext(nc) as tc:
        with tc.tile_pool(name="singles", bufs=1) as singles, \
             tc.tile_pool(name="xp", bufs=4) as xpool, \
             tc.tile_pool(name="dscr", bufs=2) as dscr, \
             tc.tile_pool(name="mscr", bufs=2) as mscr, \
             tc.tile_pool(name="pscr", bufs=2) as pscr, \
             tc.tile_pool(name="ascr", bufs=2) as ascr, \
             tc.tile_pool(name="small", bufs=4) as small:

            # persistent accumulators
            pairacc = singles.tile([_P, T * npair], f32)  # c_lo + 4096*c_hi
            sgnacc = singles.tile([_P, T * na], f32)      # ACT sign sums
            c_all = singles.tile([_P, T * 11], f32)       # c_0..c_10 per tile
            fracs = singles.tile([_P, 9], f32)            # k/10
            nfracs = singles.tile([_P, 9], f32)           # -k/10
            ebias = singles.tile([_P, 1], f32)            # -e
            c3 = c_all[:].rearrange("p (t k) -> p t k", k=11)
            nc.gpsimd.memset(c3[:, :, 0:1], float(_D))    # c_0 = 2048
            nc.gpsimd.memset(c3[:, :, 10:11], 0.0)        # c_10 = 0
            for i, f in enumerate(fr):
                nc.gpsimd.memset(fracs[:, i:i + 1], f)
                nc.gpsimd.memset(nfracs[:, i:i + 1], -f)
            nc.gpsimd.memset(ebias[:], -float(_E_F32))

            for t in range(T):
                xt = xpool.tile([_P, _D], f32, tag="xt")
                nc.sync.dma_start(out=xt[:], in_=x[t * _P:(t + 1) * _P, :])

                mx = small.tile([_P, 1], f32, tag="mx")
                nm = small.tile([_P, 1], f32, tag="nm")      # -min
                delta = small.tile([_P, 1], f32, tag="delta")
                bpos = small.tile([_P, 9], f32, tag="bpos")  # b_k
                bneg = small.tile([_P, 9], f32, tag="bneg")  # -b_k

                s_mm = dscr.tile([_P, _D], bf16, tag="dvescr")
                nc.vector.tensor_scalar(s_mm[:], xt[:], 1.0, None,
                                        Alu.mult, Alu.max, accum_out=mx[:])
                s_mm2 = dscr.tile([_P, _D], bf16, tag="dvescr")
                nc.vector.tensor_scalar(s_mm2[:], xt[:], -1.0, None,
                                        Alu.mult, Alu.max, accum_out=nm[:])
                # delta = mx + nm = mx - mn
                nc.vector.tensor_tensor(out=delta[:], in0=mx[:], in1=nm[:],
                                        op=Alu.add)
                # b_k = fl(fl(delta*frac_k) - nm)  (reference rounding order)
                nc.vector.tensor_scalar(bpos[:], fracs[:], delta[:], nm[:],
                                        Alu.mult, Alu.subtract)
                # -b_k = fl(fl(-frac_k*delta) + nm)
                nc.vector.tensor_scalar(bneg[:], nfracs[:], delta[:], nm[:],
                                        Alu.mult, Alu.add)

                for pi, (lo, hi) in enumerate(_PAIRS):
                    mhi = mscr.tile([_P, _D], f32, tag="mask")
                    nc.vector.tensor_scalar(mhi[:], xt[:], bpos[:, hi - 1:hi],
                                            4096.0, Alu.is_gt, Alu.mult)
                    sp = pscr.tile([_P, _D], f32, tag="pair")
                    col = t * npair + pi
                    nc.vector.scalar_tensor_tensor(
                        out=sp[:], in0=xt[:], scalar=bpos[:, lo - 1:lo],
                        in1=mhi[:], op0=Alu.is_gt, op1=Alu.add,
                        accum_out=pairacc[:, col:col + 1])
                for i, k in enumerate(_ACT_KS):
                    s = ascr.tile([_P, _D], bf16, tag="actscr")
                    nc.scalar.activation(
                        s[:], xt[:], Act.Sign, bias=bneg[:, k - 1:k], scale=1.0,
                        accum_out=sgnacc[:, t * na + i:t * na + i + 1])

            # ---- epilogue ----
            # unpack pairs first (DVE-only deps; runs while ACT drains)
            chi = singles.tile([_P, T * npair], f32)
            clo = singles.tile([_P, T * npair], f32)
            nc.vector.tensor_scalar(chi[:], pairacc[:], float(2.0 ** -12),
                                    _MAGIC, Alu.mult, Alu.add)
            nc.vector.tensor_scalar(chi[:], chi[:], -_MAGIC, None, Alu.add)
            nc.vector.scalar_tensor_tensor(
                out=clo[:], in0=chi[:], scalar=-4096.0, in1=pairacc[:],
                op0=Alu.mult, op1=Alu.add)
            chi3 = chi[:].rearrange("p (t k) -> p t k", k=npair)
            clo3 = clo[:].rearrange("p (t k) -> p t k", k=npair)
            for pi, (lo, hi) in enumerate(_PAIRS):
                nc.vector.tensor_copy(c3[:, :, lo:lo + 1], clo3[:, :, pi:pi + 1])
                nc.vector.tensor_copy(c3[:, :, hi:hi + 1], chi3[:, :, pi:pi + 1])
            # ACT sign-sums -> counts, scattered into c_all cols 5..9
            conv = singles.tile([_P, T * na], f32)
            nc.vector.tensor_scalar(conv[:], sgnacc[:], 0.5, float(_D // 2),
                                    Alu.mult, Alu.add)
            conv3 = conv[:].rearrange("p (t k) -> p t k", k=na)
            nc.vector.tensor_copy(c3[:, :, 5:10], conv3[:, :, 0:na])
            # obs_j = c_j - c_{j+1}
            obs = singles.tile([_P, T * 10], f32)
            obs3 = obs[:].rearrange("p (t j) -> p t j", j=10)
            nc.vector.tensor_tensor(out=obs3[:, :, 0:10], in0=c3[:, :, 0:10],
                                    in1=c3[:, :, 1:11], op=Alu.subtract)

            sq = singles.tile([_P, T * 10], f32)
            part = singles.tile([_P, 1], f32)
            nc.scalar.activation(sq[:], obs[:], Act.Square,
                                 bias=ebias[:], scale=1.0,
                                 accum_out=part[:])
            nc.sync.dma_start(out=out[:], in_=part[:])

    nc.compile()
    return nc


def _get_program():
    if "nc" not in _CACHE:
        _CACHE["nc"] = _build_program()
    return _CACHE["nc"]


def kernel(embeddings: np.ndarray) -> np.ndarray:
    from concourse.bass_utils import run_bass_kernel_spmd

    assert embeddings.shape == (_B_FULL, _D), embeddings.shape
    x = np.ascontiguousarray(embeddings, dtype=np.float32)
    nc = _get_program()
    in_maps = [
        {"x": x[c * _ROWS_PER_CORE:(c + 1) * _ROWS_PER_CORE]}
        for c in range(_N_CORES)
    ]
    res = run_bass_kernel_spmd(nc, in_maps, core_ids=list(range(_N_CORES)))
    total = np.float64(0.0)
    for r in res.results:
        total += r["partial"].astype(np.float64).sum()
    mean_chi2 = total / np.float64(_E_F32) / np.float64(_B_FULL)
    return np.float32(mean_chi2)
